# revision 31
# baseline (speedup 1.0000x reference)
"""CTC loss (keras ctc_batch_cost semantics, full-length labels) on 8 TRN2 cores.

Pure data parallel: 16 examples/core. Per core:

1. Gather (per example x 128-t chunk): contiguous 512KB y DMA, Pool
   ap_gather picks the 97 extended-label classes per t row (int16 idx
   tile, 16B-aligned per-example slices), one PE transpose puts states
   on partitions, ACT scatters K*(p+eps) into per-chain PPg buffers
   [97, u*32 + half*16 + ex], and the z half is P' * skip-mask (DVE in
   phase A, ACT during the scan to keep the DVE queue clear).  Chunks
   q=0,3 run first; q=1,2 stream + scatter during the early scan (their
   blocks are consumed only from step 128 on).

2. Bidirectional scaled prob-domain scan, two 16-example chains (F =
   forward t=0..255, B = backward t=511..256), f32-safe via per-16-step
   per-example renormalization to TGT=1e30 (scale taken from az(u-2) so
   the reciprocal chain stays off the critical path; every applied
   scale is undone exactly via accumulated ln(c) + compile-time
   constants):
     az = [alpha | alpha*mask],  alpha' = (A0@alpha + Sh2@z) * P'_t
   Per chain per step: two accumulating bf16 matmuls with constant
   banded matrices + ONE fused DVE multiply az' = rep2(PSUM) * PPg[:, u].

3. Meet: dot = sum_s (A0@alpha_255 + Sh2@z_255)[s] * q_256[s] via two
   pre-scaled paths.  All Ln inputs are clamped into the HW table's
   valid domain (~[2^-60, 2^60] -- it returns garbage above ~2^64!) and
   the path select is the cancellation-safe cond*v1 + (1-cond)*v2 (NOT
   v2 + cond*(v1-v2), which loses all mantissa bits when the unselected
   path is garbage).
"""

import sys

for p in ("/opt/trn_rl_repo", "/root/.axon_site/_ro/trn_rl_repo"):
    if p not in sys.path:
        sys.path.insert(0, p)

import numpy as np

import concourse.bass as bass
import concourse.bacc as bacc
import concourse.tile as tile
from concourse import mybir
from concourse.alu_op_type import AluOpType
from concourse.bass_utils import run_bass_kernel_spmd

F32 = mybir.dt.float32
F32R = mybir.dt.float32r
BF16 = mybir.dt.bfloat16
I16 = mybir.dt.int16
AF = mybir.ActivationFunctionType

N_CORES = 8
B, T, C, L = 128, 512, 1024, 48
S = 2 * L + 1  # 97
BC = B // N_CORES  # 16
KS = 512.0
EPS = 1e-7
TGT = 1e30
CSC = 2.0 ** -100
SCL1SQ = 2.0 ** -100
SCL2 = 2.0 ** 15
DSC1 = 2.0 ** 55
SWITCH = 1e-25
RN = 32  # renorm period
TM = T // 2  # 256
BLANK = C - 1
N_RENORM = 2 * (TM // RN - 1)  # 15 fwd + 15 bwd
_BIAS_COMMON = (
    T * np.log(KS) + (N_RENORM + 2) * np.log(TGT) + N_RENORM * np.log(CSC)
)
BIAS1 = float(_BIAS_COMMON + np.log(SCL1SQ) + np.log(DSC1))
BIAS2 = float(_BIAS_COMMON + 2 * np.log(SCL2))

_built = None


def _np_consts():
    a0t = np.zeros((S, S), np.float32)
    for k in range(S):
        a0t[k, k] = 1.0
        if k + 1 < S:
            a0t[k, k + 1] = 1.0
    sh2t = np.zeros((S, S), np.float32)
    for k in range(S - 2):
        sh2t[k, k + 2] = 1.0
    a0b = np.zeros((S, S), np.float32)
    for k in range(S):
        a0b[k, k] = 1.0
        if k - 1 >= 0:
            a0b[k, k - 1] = 1.0
    sh2b = np.zeros((S, S), np.float32)
    for k in range(2, S):
        sh2b[k, k - 2] = 1.0
    e01t = np.zeros((S, 2 * BC), np.float32)
    e01t[0:2, :] = TGT
    e96t = np.zeros((S, 2 * BC), np.float32)
    e96t[S - 2 :, :] = TGT
    ones_col = np.ones((S, 1), np.float32)
    tcol = np.full((1, S), TGT, np.float32)
    ident = np.eye(128, dtype=np.float32)
    return {
        "a0t": a0t, "sh2t": sh2t, "a0b": a0b, "sh2b": sh2b,
        "e01t": e01t, "e96t": e96t,
        "ones_col": ones_col, "tcol": tcol, "ident": ident,
    }


def _build():
    global _built
    if _built is not None:
        return _built

    cdefs = _np_consts()

    nc = bacc.Bacc("TRN2")
    y = nc.dram_tensor("y_pred", [BC, T, C], F32, kind="ExternalInput")
    exti = nc.dram_tensor("ext_i16", [128, BC * 8], I16, kind="ExternalInput")
    gf = nc.dram_tensor("gf", [S, BC], F32, kind="ExternalInput")
    gsm = nc.dram_tensor("gsm", [S, BC], F32, kind="ExternalInput")
    loss = nc.dram_tensor("loss", [BC, 1], F32, kind="ExternalOutput")

    handles = {k: nc.inline_tensor(v, name=f"{k}_c") for k, v in cdefs.items()}

    with tile.TileContext(nc) as tc:
        with (
            tc.tile_pool(name="consts", bufs=1) as consts,
            tc.tile_pool(name="persist", bufs=1) as persist,
            tc.tile_pool(name="ybufs", bufs=3) as ybufs,
            tc.tile_pool(name="gthp", bufs=36) as gthp,
            tc.tile_pool(name="idxp", bufs=2) as idxp,
            tc.tile_pool(name="azp", bufs=4) as azp,
            tc.tile_pool(name="small", bufs=4) as small,
            tc.tile_pool(name="tr_ps", bufs=3, space="PSUM") as tr_ps,
            tc.tile_pool(name="sc_ps", bufs=1, space="PSUM") as sc_ps,
            tc.tile_pool(name="rn_ps", bufs=1, space="PSUM") as rn_ps,
        ):
            cs = {}
            for k, v in cdefs.items():
                cs[k] = consts.tile(list(v.shape), F32, tag=k, name=f"c_{k}")
                nc.sync.dma_start(out=cs[k], in_=handles[k].ap())
            csb = {}
            for k in ("a0t", "sh2t", "a0b", "sh2b", "ones_col"):
                csb[k] = consts.tile(
                    list(cdefs[k].shape), BF16, tag=f"{k}_bf", name=f"cb_{k}"
                )
                nc.scalar.activation(
                    out=csb[k], in_=cs[k], func=AF.Copy, scale=1.0
                )
            masks = {}
            for k, h in (("gf", gf), ("gsm", gsm)):
                masks[k] = consts.tile([S, BC], F32, tag=k, name=f"m_{k}")
                nc.sync.dma_start(out=masks[k], in_=h.ap())

            # PPg buffers: chains F, B -> [97, TM * 32]
            # block u columns: [0:16] = P', [16:32] = P'*mask
            ppg = {
                ch: persist.tile(
                    [S, TM * 2 * BC], F32, tag=f"ppg_{ch}", name=f"ppg_{ch}"
                )
                for ch in ("F", "B")
            }

            y_ap = y.ap()

            # ---------------- gather ----------------
            # Unit (b, q): contiguous 512KB y DMA, Pool ap_gather (112 idxs),
            # one PE transpose to put states on partitions, one ACT strided
            # scatter into PPg (folding K scale + K*eps bias), one DVE
            # mask-mult for the z half.  Chunks q=0,3 run first; q=1,2 are
            # interleaved into the scan (their blocks are needed only from
            # step 128 on).
            idx_all = consts.tile([128, BC * 8], I16, tag="idx_all", name="idx_all")
            nc.sync.dma_start(out=idx_all, in_=exti.ap())

            def dst_ap(buf, col):
                return bass.AP(
                    tensor=buf.tensor,
                    offset=buf.offset + col,
                    ap=[buf.ap[0], [2 * BC, 128]],
                )

            _dma_rr = [0]

            def gather_dma(b, q):
                # round-robin 4 tags (bufs=4 each): 4 independent semaphore
                # streams keep more DMAs in flight than one 16-deep ring
                k = _dma_rr[0] % 8
                _dma_rr[0] += 1
                yt = ybufs.tile([128, C], F32, tag=f"y{k}", name="y_t")
                nc.sync.dma_start(out=yt, in_=y_ap[b, q * 128 : (q + 1) * 128, :])
                return yt

            def gather_gather(b, q, yt):
                gth = gthp.tile([128, 112], F32, tag="gth", name="gth")
                nc.gpsimd.ap_gather(
                    out_ap=gth, in_ap=yt, idxs_ap=idx_all[:, b * 8 : b * 8 + 7],
                    channels=128, num_elems=C, d=1, num_idxs=112,
                )
                return gth

            def gather_scatter(b, q, gth, z_on_dve):
                ptr = tr_ps.tile([S, 128], F32, tag="tr", name="ptr")
                nc.tensor.transpose(ptr, gth[:, 0:S], cs["ident"])
                if q < 2:
                    buf, mk, blk0 = ppg["F"], masks["gf"], q * 128
                else:
                    buf, mk, blk0 = ppg["B"], masks["gsm"], (q - 2) * 128
                d_a = dst_ap(buf, blk0 * 2 * BC + b)
                nc.scalar.activation(
                    out=d_a, in_=ptr, func=AF.Copy, scale=KS, bias=KS * EPS
                )
                # z-half: z = mask * P' (mask is a per-partition 0/1 column).
                # DVE in phase A (idle there); ACT during the scan (keeps the
                # DVE queue clear for scan TTs).
                z_dst = dst_ap(buf, blk0 * 2 * BC + BC + b)
                if z_on_dve:
                    nc.vector.tensor_scalar(
                        out=z_dst, in0=d_a, scalar1=mk[:, b : b + 1],
                        scalar2=None, op0=AluOpType.mult,
                    )
                else:
                    nc.scalar.activation(
                        out=z_dst, in_=d_a, func=AF.Copy,
                        scale=mk[:, b : b + 1],
                    )

            phase_a = [(b, q) for q in (0, 3) for b in range(BC)]
            a_fifo = [(b, q, gather_dma(b, q)) for b, q in phase_a]
            for b, q, yt in a_fifo:
                gather_scatter(b, q, gather_gather(b, q, yt), z_on_dve=True)

            # ---------------- scan ----------------
            chains = ["F", "B"]
            mats = {"F": ("a0t", "sh2t"), "B": ("a0b", "sh2b")}

            def sl(ch, u):
                idx = u if ch == "F" else TM - 1 - u
                return idx * 2 * BC

            az, lnacc = {}, {}
            for ch in chains:
                a0 = azp.tile([S, 2 * BC], BF16, tag=f"az{ch}", name=f"az_{ch}")
                init_mask = cs["e01t"] if ch == "F" else cs["e96t"]
                nc.vector.tensor_tensor(
                    out=a0,
                    in0=ppg[ch][:, sl(ch, 0) : sl(ch, 0) + 2 * BC],
                    in1=init_mask,
                    op=AluOpType.mult,
                )
                az[ch] = a0
                ln0 = small.tile([1, BC], F32, tag=f"ln{ch}", name=f"ln_{ch}")
                nc.vector.memset(ln0, 0.0)
                lnacc[ch] = ln0

            # hoisted renorm state: ppgs tiles prepared two steps early
            pending = {}
            rem_units = [(b, q) for q in (1, 2) for b in range(BC)]
            # q1/q2: DMAs burst-issued now (ybufs ring paces, ~16 in flight);
            # Pool ap_gathers sprinkled over idle early-scan steps; the PE
            # transposes + ACT scatters run as two compact bursts so the PE
            # pipeline restarts only twice.
            inflight = [(b, q, gather_dma(b, q)) for b, q in rem_units]
            gths = [gather_gather(b_, q_, yt_) for b_, q_, yt_ in inflight]
            sc_cur = 0
            for u in range(1, TM):
                if sc_cur < len(inflight) and u % 4 == 1:
                    b_, q_, _ = inflight[sc_cur]
                    gather_scatter(b_, q_, gths[sc_cur], z_on_dve=False)
                    sc_cur += 1
                for ch in chains:
                    # ---- this chain's step u ----
                    if u % RN == 0:
                        ppg_in = pending.pop((ch, u))
                    else:
                        o = sl(ch, u)
                        ppg_in = ppg[ch][:, o : o + 2 * BC]
                    psx = sc_ps.tile([S, BC], F32, tag=f"ps{ch}", name=f"ps_{ch}")
                    nc.tensor.matmul(
                        psx, csb[mats[ch][0]], az[ch][:, 0:BC],
                        start=True, stop=False,
                    )
                    nc.tensor.matmul(
                        psx, csb[mats[ch][1]], az[ch][:, BC : 2 * BC],
                        start=False, stop=True,
                    )
                    nw = azp.tile([S, 2 * BC], BF16, tag=f"az{ch}", name=f"aznw_{ch}")
                    rep = bass.AP(
                        tensor=psx.tensor,
                        offset=psx.offset,
                        ap=[psx.ap[0], [0, 2], [1, BC]],
                    )
                    nc.vector.tensor_tensor(
                        out=nw.rearrange("p (r c) -> p r c", r=2),
                        in0=rep,
                        in1=ppg_in.rearrange("p (r c) -> p r c", r=2),
                        op=AluOpType.mult,
                    )
                    az[ch] = nw

                    # ---- prepare renorm for step u+2 from az(u) (stale-ok) ----
                    ur = u + 2
                    if ur % RN == 0 and ur < TM:
                        c_ps = rn_ps.tile([1, BC], F32, tag=f"rn{ch}", name=f"cps_{ch}")
                        nc.tensor.matmul(
                            c_ps, csb["ones_col"], az[ch][:, 0:BC],
                            start=True, stop=True,
                        )
                        r_sb = small.tile([1, BC], F32, tag=f"r{ch}", name=f"r_{ch}")
                        nc.vector.reciprocal(r_sb, c_ps)
                        lnc = small.tile([1, BC], F32, tag=f"lnc{ch}", name=f"lnc_{ch}")
                        nc.scalar.activation(out=lnc, in_=c_ps, func=AF.Ln, scale=CSC)
                        ln_new = small.tile([1, BC], F32, tag=f"ln{ch}", name=f"lnn_{ch}")
                        nc.vector.tensor_tensor(
                            out=ln_new, in0=lnacc[ch], in1=lnc, op=AluOpType.add
                        )
                        lnacc[ch] = ln_new
                        rr_ps = rn_ps.tile([S, BC], F32, tag=f"rn{ch}", name=f"rr_{ch}")
                        nc.tensor.matmul(
                            rr_ps, cs["tcol"], r_sb, start=True, stop=True
                        )
                        ppgs = small.tile([S, 2, BC], F32, tag=f"ppgs{ch}", name=f"ppgs_{ch}")
                        rr_rep = bass.AP(
                            tensor=rr_ps.tensor,
                            offset=rr_ps.offset,
                            ap=[rr_ps.ap[0], [0, 2], [1, BC]],
                        )
                        o = sl(ch, ur)
                        nc.vector.tensor_tensor(
                            out=ppgs,
                            in0=ppg[ch][:, o : o + 2 * BC].rearrange(
                                "p (r c) -> p r c", r=2
                            ),
                            in1=rr_rep,
                            op=AluOpType.mult,
                        )
                        pending[(ch, ur)] = ppgs.rearrange("p r c -> p (r c)")

            # ---------------- meet + finalize ----------------
            comb = sc_ps.tile([S, BC], F32, tag="psF", name="comb")
            nc.tensor.matmul(
                comb, csb["a0t"], az["F"][:, 0:BC], start=True, stop=False
            )
            nc.tensor.matmul(
                comb, csb["sh2t"], az["F"][:, BC : 2 * BC], start=False, stop=True
            )
            qv = az["B"][:, 0:BC]
            # path 1 (shallow): product scaled by 2^-100 (fold onto q)
            q1 = small.tile([S, BC], F32, tag="q1")
            nc.vector.tensor_scalar(
                out=q1, in0=qv, scalar1=SCL1SQ, scalar2=None, op0=AluOpType.mult
            )
            m1 = small.tile([S, BC], BF16, tag="m1")
            nc.vector.tensor_tensor(out=m1, in0=comb, in1=q1, op=AluOpType.mult)
            dot1 = sc_ps.tile([1, BC], F32, tag="psB", name="dot1")
            nc.tensor.matmul(dot1, csb["ones_col"], m1, start=True, stop=True)
            # path 2 (deep): each factor scaled by 2^15 and clamped
            q2 = small.tile([S, BC], F32, tag="q2")
            nc.vector.tensor_scalar(
                out=q2, in0=qv, scalar1=SCL2, scalar2=None, op0=AluOpType.mult
            )
            c2 = small.tile([S, BC], F32, tag="c2")
            nc.vector.tensor_scalar(
                out=c2, in0=comb, scalar1=SCL2, scalar2=None, op0=AluOpType.mult
            )
            m2 = small.tile([S, BC], F32, tag="m2")
            nc.vector.tensor_tensor(out=m2, in0=c2, in1=q2, op=AluOpType.mult)
            m2c = small.tile([S, BC], BF16, tag="m2c")
            nc.vector.tensor_scalar(
                out=m2c, in0=m2, scalar1=1e37, scalar2=None, op0=AluOpType.min
            )
            dot2 = sc_ps.tile([1, BC], F32, tag="psB", name="dot2")
            nc.tensor.matmul(dot2, csb["ones_col"], m2c, start=True, stop=True)

            # clamp both dots so the Ln table input stays in ~[2^-60, 2^60]
            d1c = small.tile([1, BC], F32, tag="d1c")
            nc.vector.tensor_scalar(
                out=d1c, in0=dot1, scalar1=1e-30, scalar2=1e-4,
                op0=AluOpType.max, op1=AluOpType.min,
            )
            d2c = small.tile([1, BC], F32, tag="d2c")
            nc.vector.tensor_scalar(
                out=d2c, in0=dot2, scalar1=1e-17, scalar2=3e15,
                op0=AluOpType.max, op1=AluOpType.min,
            )
            lnd1 = small.tile([1, BC], F32, tag="lnd1")
            nc.scalar.activation(out=lnd1, in_=d1c, func=AF.Ln, scale=DSC1)
            lnd2 = small.tile([1, BC], F32, tag="lnd2")
            nc.scalar.activation(out=lnd2, in_=d2c, func=AF.Ln, scale=1.0)
            cond = small.tile([1, BC], F32, tag="cond")
            nc.vector.tensor_scalar(
                out=cond, in0=d1c, scalar1=SWITCH, scalar2=None,
                op0=AluOpType.is_gt,
            )
            condn = small.tile([1, BC], F32, tag="condn")
            nc.vector.tensor_scalar(
                out=condn, in0=d1c, scalar1=SWITCH, scalar2=None,
                op0=AluOpType.is_le,
            )
            lnT = small.tile([1, BC], F32, tag="lnT")
            nc.vector.tensor_tensor(
                out=lnT, in0=lnacc["F"], in1=lnacc["B"], op=AluOpType.add
            )
            u1 = small.tile([1, BC], F32, tag="u1")
            nc.vector.tensor_tensor(out=u1, in0=lnd1, in1=lnT, op=AluOpType.add)
            u2 = small.tile([1, BC], F32, tag="u2")
            nc.vector.tensor_tensor(out=u2, in0=lnd2, in1=lnT, op=AluOpType.add)
            v1 = small.tile([1, BC], F32, tag="v1")
            nc.scalar.activation(out=v1, in_=u1, func=AF.Copy, scale=-1.0, bias=BIAS1)
            v2 = small.tile([1, BC], F32, tag="v2")
            nc.scalar.activation(out=v2, in_=u2, func=AF.Copy, scale=-1.0, bias=BIAS2)
            # cancellation-safe select: cond*v1 + (1-cond)*v2
            p1 = small.tile([1, BC], F32, tag="p1")
            nc.vector.tensor_tensor(out=p1, in0=cond, in1=v1, op=AluOpType.mult)
            p2 = small.tile([1, BC], F32, tag="p2")
            nc.vector.tensor_tensor(out=p2, in0=condn, in1=v2, op=AluOpType.mult)
            lsb = small.tile([1, BC], F32, tag="lsb")
            nc.vector.tensor_tensor(out=lsb, in0=p1, in1=p2, op=AluOpType.add)
            nc.sync.dma_start(out=loss.ap().rearrange("b o -> o b"), in_=lsb)

    nc.compile()
    _built = nc
    return nc


def _host_prep(y_pred: np.ndarray, y_true: np.ndarray):
    y_true = np.asarray(y_true).astype(np.int64)
    ext = np.full((B, S), BLANK, np.int64)
    ext[:, 1::2] = y_true
    ext_m2 = np.concatenate([np.full((B, 2), -1, np.int64), ext[:, :-2]], axis=1)
    skip = (ext != BLANK) & (ext != ext_m2)  # [B,S]
    g = np.zeros((B, S), np.float32)
    g[:, :-2] = skip[:, 2:].astype(np.float32)  # fwd z-mask: g[s] = skip[s+2]
    sm = skip.astype(np.float32)  # bwd w-mask

    # ap_gather indices: 112 per example (97 labels + pad 0), wrapped so
    # index i sits at partition i%16, col i//16, replicated across the 8
    # groups of 16 partitions; examples side by side -> [128, BC*7] per core.
    idxs = np.zeros((B, 112), np.int16)
    idxs[:, :S] = ext.astype(np.int16)
    wrapped = np.zeros((B, 16, 7), np.int16)
    ii = np.arange(112)
    wrapped[:, ii % 16, ii // 16] = idxs
    tiled = np.tile(wrapped, (1, 8, 1))  # [B, 128, 7]

    in_maps = []
    for core in range(N_CORES):
        sl_ = slice(core * BC, (core + 1) * BC)
        in_maps.append(
            {
                "y_pred": np.ascontiguousarray(y_pred[sl_]).astype(np.float32),
                "ext_i16": np.ascontiguousarray(
                    np.concatenate(
                        [tiled[sl_], np.zeros((BC, 128, 1), np.int16)], axis=2
                    ).transpose(1, 0, 2).reshape(128, BC * 8)
                ),
                "gf": np.ascontiguousarray(g[sl_].T),
                "gsm": np.ascontiguousarray(sm[sl_].T),
            }
        )
    return in_maps


def kernel(y_pred: np.ndarray, y_true: np.ndarray) -> np.ndarray:
    nc = _build()
    in_maps = _host_prep(y_pred, y_true)
    res = run_bass_kernel_spmd(nc, in_maps, core_ids=list(range(N_CORES)))
    out = np.concatenate([r["loss"] for r in res.results], axis=0)
    return out.astype(np.float32)


# revision 32
# speedup vs baseline: 1.0022x; 1.0022x over previous
"""CTC loss (keras ctc_batch_cost semantics, full-length labels) on 8 TRN2 cores.

Pure data parallel: 16 examples/core. Per core:

1. Gather (per example x 128-t chunk): contiguous 512KB y DMA, Pool
   ap_gather picks the 97 extended-label classes per t row (int16 idx
   tile, 16B-aligned per-example slices), one PE transpose puts states
   on partitions, ACT scatters K*(p+eps) into per-chain PPg buffers
   [97, u*32 + half*16 + ex], and the z half is P' * skip-mask (DVE in
   phase A, ACT during the scan to keep the DVE queue clear).  Chunks
   q=0,3 run first; q=1,2 stream + scatter during the early scan (their
   blocks are consumed only from step 128 on).

2. Bidirectional scaled prob-domain scan, two 16-example chains (F =
   forward t=0..255, B = backward t=511..256), f32-safe via per-32-step
   per-example renormalization to TGT=1e30 (scale taken from az(u-2) so
   the reciprocal chain stays off the critical path; every applied
   scale is undone exactly via accumulated ln(c) + compile-time
   constants):
     az = [alpha | alpha*mask],  alpha' = (A0@alpha + Sh2@z) * P'_t
   Per chain per step: two accumulating bf16 matmuls with constant
   banded matrices + ONE fused DVE multiply az' = rep2(PSUM) * PPg[:, u].

3. Meet: dot = sum_s (A0@alpha_255 + Sh2@z_255)[s] * q_256[s] via two
   pre-scaled paths.  All Ln inputs are clamped into the HW table's
   valid domain (~[2^-60, 2^60] -- it returns garbage above ~2^64!) and
   the path select is the cancellation-safe cond*v1 + (1-cond)*v2 (NOT
   v2 + cond*(v1-v2), which loses all mantissa bits when the unselected
   path is garbage).
"""

import sys

for p in ("/opt/trn_rl_repo", "/root/.axon_site/_ro/trn_rl_repo"):
    if p not in sys.path:
        sys.path.insert(0, p)

import numpy as np

import concourse.bass as bass
import concourse.bacc as bacc
import concourse.tile as tile
from concourse import mybir
from concourse.alu_op_type import AluOpType
from concourse.bass_utils import run_bass_kernel_spmd

F32 = mybir.dt.float32
F32R = mybir.dt.float32r
BF16 = mybir.dt.bfloat16
I16 = mybir.dt.int16
AF = mybir.ActivationFunctionType

N_CORES = 8
B, T, C, L = 128, 512, 1024, 48
S = 2 * L + 1  # 97
BC = B // N_CORES  # 16
KS = 512.0
EPS = 1e-7
TGT = 1e30
CSC = 2.0 ** -100
SCL1SQ = 2.0 ** -100
SCL2 = 2.0 ** 15
DSC1 = 2.0 ** 55
SWITCH = 1e-25
RN = 32  # renorm period
TM = T // 2  # 256
BLANK = C - 1
N_RENORM = 2 * (TM // RN - 1)  # 7 fwd + 7 bwd
_BIAS_COMMON = (
    T * np.log(KS) + (N_RENORM + 2) * np.log(TGT) + N_RENORM * np.log(CSC)
)
BIAS1 = float(_BIAS_COMMON + np.log(SCL1SQ) + np.log(DSC1))
BIAS2 = float(_BIAS_COMMON + 2 * np.log(SCL2))

_built = None


def _np_consts():
    a0t = np.zeros((S, S), np.float32)
    for k in range(S):
        a0t[k, k] = 1.0
        if k + 1 < S:
            a0t[k, k + 1] = 1.0
    sh2t = np.zeros((S, S), np.float32)
    for k in range(S - 2):
        sh2t[k, k + 2] = 1.0
    a0b = np.zeros((S, S), np.float32)
    for k in range(S):
        a0b[k, k] = 1.0
        if k - 1 >= 0:
            a0b[k, k - 1] = 1.0
    sh2b = np.zeros((S, S), np.float32)
    for k in range(2, S):
        sh2b[k, k - 2] = 1.0
    e01t = np.zeros((S, 2 * BC), np.float32)
    e01t[0:2, :] = TGT
    e96t = np.zeros((S, 2 * BC), np.float32)
    e96t[S - 2 :, :] = TGT
    ones_col = np.ones((S, 1), np.float32)
    tcol = np.full((1, S), TGT, np.float32)
    ident = np.eye(128, dtype=np.float32)
    return {
        "a0t": a0t, "sh2t": sh2t, "a0b": a0b, "sh2b": sh2b,
        "e01t": e01t, "e96t": e96t,
        "ones_col": ones_col, "tcol": tcol, "ident": ident,
    }


def _build():
    global _built
    if _built is not None:
        return _built

    cdefs = _np_consts()

    nc = bacc.Bacc("TRN2")
    y = nc.dram_tensor("y_pred", [BC, T, C], F32, kind="ExternalInput")
    exti = nc.dram_tensor("ext_i16", [128, BC * 8], I16, kind="ExternalInput")
    gf = nc.dram_tensor("gf", [S, BC], F32, kind="ExternalInput")
    gsm = nc.dram_tensor("gsm", [S, BC], F32, kind="ExternalInput")
    loss = nc.dram_tensor("loss", [BC, 1], F32, kind="ExternalOutput")

    handles = {k: nc.inline_tensor(v, name=f"{k}_c") for k, v in cdefs.items()}

    with tile.TileContext(nc) as tc:
        with (
            tc.tile_pool(name="consts", bufs=1) as consts,
            tc.tile_pool(name="persist", bufs=1) as persist,
            tc.tile_pool(name="ybufs", bufs=3) as ybufs,
            tc.tile_pool(name="gthp", bufs=36) as gthp,
            tc.tile_pool(name="idxp", bufs=2) as idxp,
            tc.tile_pool(name="azp", bufs=4) as azp,
            tc.tile_pool(name="small", bufs=4) as small,
            tc.tile_pool(name="tr_ps", bufs=3, space="PSUM") as tr_ps,
            tc.tile_pool(name="sc_ps", bufs=1, space="PSUM") as sc_ps,
            tc.tile_pool(name="rn_ps", bufs=1, space="PSUM") as rn_ps,
        ):
            cs = {}
            for k, v in cdefs.items():
                cs[k] = consts.tile(list(v.shape), F32, tag=k, name=f"c_{k}")
                nc.sync.dma_start(out=cs[k], in_=handles[k].ap())
            csb = {}
            for k in ("a0t", "sh2t", "a0b", "sh2b", "ones_col"):
                csb[k] = consts.tile(
                    list(cdefs[k].shape), BF16, tag=f"{k}_bf", name=f"cb_{k}"
                )
                nc.scalar.activation(
                    out=csb[k], in_=cs[k], func=AF.Copy, scale=1.0
                )
            masks = {}
            for k, h in (("gf", gf), ("gsm", gsm)):
                masks[k] = consts.tile([S, BC], F32, tag=k, name=f"m_{k}")
                nc.sync.dma_start(out=masks[k], in_=h.ap())

            # PPg buffers: chains F, B -> [97, TM * 32]
            # block u columns: [0:16] = P', [16:32] = P'*mask
            ppg = {
                ch: persist.tile(
                    [S, TM * 2 * BC], F32, tag=f"ppg_{ch}", name=f"ppg_{ch}"
                )
                for ch in ("F", "B")
            }

            y_ap = y.ap()

            # ---------------- gather ----------------
            # Unit (b, q): contiguous 512KB y DMA, Pool ap_gather (112 idxs),
            # one PE transpose to put states on partitions, one ACT strided
            # scatter into PPg (folding K scale + K*eps bias), one DVE
            # mask-mult for the z half.  Chunks q=0,3 run first; q=1,2 are
            # interleaved into the scan (their blocks are needed only from
            # step 128 on).
            idx_all = consts.tile([128, BC * 8], I16, tag="idx_all", name="idx_all")
            nc.sync.dma_start(out=idx_all, in_=exti.ap())

            def dst_ap(buf, col):
                return bass.AP(
                    tensor=buf.tensor,
                    offset=buf.offset + col,
                    ap=[buf.ap[0], [2 * BC, 128]],
                )

            _dma_rr = [0]

            def gather_dma(b, q):
                # round-robin 4 tags (bufs=4 each): 4 independent semaphore
                # streams keep more DMAs in flight than one 16-deep ring
                k = _dma_rr[0] % 8
                _dma_rr[0] += 1
                yt = ybufs.tile([128, C], F32, tag=f"y{k}", name="y_t")
                nc.sync.dma_start(out=yt, in_=y_ap[b, q * 128 : (q + 1) * 128, :])
                return yt

            def gather_gather(b, q, yt):
                gth = gthp.tile([128, 112], F32, tag="gth", name="gth")
                nc.gpsimd.ap_gather(
                    out_ap=gth, in_ap=yt, idxs_ap=idx_all[:, b * 8 : b * 8 + 7],
                    channels=128, num_elems=C, d=1, num_idxs=112,
                )
                return gth

            def gather_scatter(b, q, gth, z_on_dve):
                ptr = tr_ps.tile([S, 128], F32, tag="tr", name="ptr")
                nc.tensor.transpose(ptr, gth[:, 0:S], cs["ident"])
                if q < 2:
                    buf, mk, blk0 = ppg["F"], masks["gf"], q * 128
                else:
                    buf, mk, blk0 = ppg["B"], masks["gsm"], (q - 2) * 128
                d_a = dst_ap(buf, blk0 * 2 * BC + b)
                nc.scalar.activation(
                    out=d_a, in_=ptr, func=AF.Copy, scale=KS, bias=KS * EPS
                )
                # z-half: z = mask * P' (mask is a per-partition 0/1 column).
                # DVE in phase A (idle there); ACT during the scan (keeps the
                # DVE queue clear for scan TTs).
                z_dst = dst_ap(buf, blk0 * 2 * BC + BC + b)
                if z_on_dve:
                    nc.vector.tensor_scalar(
                        out=z_dst, in0=d_a, scalar1=mk[:, b : b + 1],
                        scalar2=None, op0=AluOpType.mult,
                    )
                else:
                    nc.scalar.activation(
                        out=z_dst, in_=d_a, func=AF.Copy,
                        scale=mk[:, b : b + 1],
                    )

            phase_a = [(b, q) for q in (0, 3) for b in range(BC)]
            a_fifo = [(b, q, gather_dma(b, q)) for b, q in phase_a]
            for b, q, yt in a_fifo:
                gather_scatter(b, q, gather_gather(b, q, yt), z_on_dve=True)

            # ---------------- scan ----------------
            chains = ["F", "B"]
            mats = {"F": ("a0t", "sh2t"), "B": ("a0b", "sh2b")}

            def sl(ch, u):
                idx = u if ch == "F" else TM - 1 - u
                return idx * 2 * BC

            az, lnacc = {}, {}
            for ch in chains:
                a0 = azp.tile([S, 2 * BC], BF16, tag=f"az{ch}", name=f"az_{ch}")
                init_mask = cs["e01t"] if ch == "F" else cs["e96t"]
                nc.vector.tensor_tensor(
                    out=a0,
                    in0=ppg[ch][:, sl(ch, 0) : sl(ch, 0) + 2 * BC],
                    in1=init_mask,
                    op=AluOpType.mult,
                )
                az[ch] = a0
                ln0 = small.tile([1, BC], F32, tag=f"ln{ch}", name=f"ln_{ch}")
                nc.vector.memset(ln0, 0.0)
                lnacc[ch] = ln0

            # hoisted renorm state: ppgs tiles prepared two steps early
            pending = {}
            rem_units = [(b, q) for q in (1, 2) for b in range(BC)]
            # q1/q2: DMAs burst-issued now (ybufs ring paces, ~16 in flight);
            # Pool ap_gathers sprinkled over idle early-scan steps; the PE
            # transposes + ACT scatters run as two compact bursts so the PE
            # pipeline restarts only twice.
            inflight = [(b, q, gather_dma(b, q)) for b, q in rem_units]
            gths = [gather_gather(b_, q_, yt_) for b_, q_, yt_ in inflight]
            sc_cur = 0
            for u in range(1, TM):
                if sc_cur < len(inflight) and (u % 3 == 1 or u >= 100):
                    b_, q_, _ = inflight[sc_cur]
                    gather_scatter(b_, q_, gths[sc_cur], z_on_dve=False)
                    sc_cur += 1
                for ch in chains:
                    # ---- this chain's step u ----
                    if u % RN == 0:
                        ppg_in = pending.pop((ch, u))
                    else:
                        o = sl(ch, u)
                        ppg_in = ppg[ch][:, o : o + 2 * BC]
                    psx = sc_ps.tile([S, BC], F32, tag=f"ps{ch}", name=f"ps_{ch}")
                    nc.tensor.matmul(
                        psx, csb[mats[ch][0]], az[ch][:, 0:BC],
                        start=True, stop=False,
                    )
                    nc.tensor.matmul(
                        psx, csb[mats[ch][1]], az[ch][:, BC : 2 * BC],
                        start=False, stop=True,
                    )
                    nw = azp.tile([S, 2 * BC], BF16, tag=f"az{ch}", name=f"aznw_{ch}")
                    rep = bass.AP(
                        tensor=psx.tensor,
                        offset=psx.offset,
                        ap=[psx.ap[0], [0, 2], [1, BC]],
                    )
                    nc.vector.tensor_tensor(
                        out=nw.rearrange("p (r c) -> p r c", r=2),
                        in0=rep,
                        in1=ppg_in.rearrange("p (r c) -> p r c", r=2),
                        op=AluOpType.mult,
                    )
                    az[ch] = nw

                    # ---- prepare renorm for step u+2 from az(u) (stale-ok) ----
                    ur = u + 2
                    if ur % RN == 0 and ur < TM:
                        c_ps = rn_ps.tile([1, BC], F32, tag=f"rn{ch}", name=f"cps_{ch}")
                        nc.tensor.matmul(
                            c_ps, csb["ones_col"], az[ch][:, 0:BC],
                            start=True, stop=True,
                        )
                        r_sb = small.tile([1, BC], F32, tag=f"r{ch}", name=f"r_{ch}")
                        nc.vector.reciprocal(r_sb, c_ps)
                        lnc = small.tile([1, BC], F32, tag=f"lnc{ch}", name=f"lnc_{ch}")
                        nc.scalar.activation(out=lnc, in_=c_ps, func=AF.Ln, scale=CSC)
                        ln_new = small.tile([1, BC], F32, tag=f"ln{ch}", name=f"lnn_{ch}")
                        nc.vector.tensor_tensor(
                            out=ln_new, in0=lnacc[ch], in1=lnc, op=AluOpType.add
                        )
                        lnacc[ch] = ln_new
                        rr_ps = rn_ps.tile([S, BC], F32, tag=f"rn{ch}", name=f"rr_{ch}")
                        nc.tensor.matmul(
                            rr_ps, cs["tcol"], r_sb, start=True, stop=True
                        )
                        ppgs = small.tile([S, 2, BC], F32, tag=f"ppgs{ch}", name=f"ppgs_{ch}")
                        rr_rep = bass.AP(
                            tensor=rr_ps.tensor,
                            offset=rr_ps.offset,
                            ap=[rr_ps.ap[0], [0, 2], [1, BC]],
                        )
                        o = sl(ch, ur)
                        nc.vector.tensor_tensor(
                            out=ppgs,
                            in0=ppg[ch][:, o : o + 2 * BC].rearrange(
                                "p (r c) -> p r c", r=2
                            ),
                            in1=rr_rep,
                            op=AluOpType.mult,
                        )
                        pending[(ch, ur)] = ppgs.rearrange("p r c -> p (r c)")

            # ---------------- meet + finalize ----------------
            comb = sc_ps.tile([S, BC], F32, tag="psF", name="comb")
            nc.tensor.matmul(
                comb, csb["a0t"], az["F"][:, 0:BC], start=True, stop=False
            )
            nc.tensor.matmul(
                comb, csb["sh2t"], az["F"][:, BC : 2 * BC], start=False, stop=True
            )
            qv = az["B"][:, 0:BC]
            # path 1 (shallow): product scaled by 2^-100 (fold onto q)
            q1 = small.tile([S, BC], F32, tag="q1")
            nc.vector.tensor_scalar(
                out=q1, in0=qv, scalar1=SCL1SQ, scalar2=None, op0=AluOpType.mult
            )
            m1 = small.tile([S, BC], BF16, tag="m1")
            nc.vector.tensor_tensor(out=m1, in0=comb, in1=q1, op=AluOpType.mult)
            dot1 = sc_ps.tile([1, BC], F32, tag="psB", name="dot1")
            nc.tensor.matmul(dot1, csb["ones_col"], m1, start=True, stop=True)
            # path 2 (deep): each factor scaled by 2^15 and clamped
            q2 = small.tile([S, BC], F32, tag="q2")
            nc.vector.tensor_scalar(
                out=q2, in0=qv, scalar1=SCL2, scalar2=None, op0=AluOpType.mult
            )
            c2 = small.tile([S, BC], F32, tag="c2")
            nc.vector.tensor_scalar(
                out=c2, in0=comb, scalar1=SCL2, scalar2=None, op0=AluOpType.mult
            )
            m2 = small.tile([S, BC], F32, tag="m2")
            nc.vector.tensor_tensor(out=m2, in0=c2, in1=q2, op=AluOpType.mult)
            m2c = small.tile([S, BC], BF16, tag="m2c")
            nc.vector.tensor_scalar(
                out=m2c, in0=m2, scalar1=1e37, scalar2=None, op0=AluOpType.min
            )
            dot2 = sc_ps.tile([1, BC], F32, tag="psB", name="dot2")
            nc.tensor.matmul(dot2, csb["ones_col"], m2c, start=True, stop=True)

            # clamp both dots so the Ln table input stays in ~[2^-60, 2^60]
            d1c = small.tile([1, BC], F32, tag="d1c")
            nc.vector.tensor_scalar(
                out=d1c, in0=dot1, scalar1=1e-30, scalar2=1e-4,
                op0=AluOpType.max, op1=AluOpType.min,
            )
            d2c = small.tile([1, BC], F32, tag="d2c")
            nc.vector.tensor_scalar(
                out=d2c, in0=dot2, scalar1=1e-17, scalar2=3e15,
                op0=AluOpType.max, op1=AluOpType.min,
            )
            lnd1 = small.tile([1, BC], F32, tag="lnd1")
            nc.scalar.activation(out=lnd1, in_=d1c, func=AF.Ln, scale=DSC1)
            lnd2 = small.tile([1, BC], F32, tag="lnd2")
            nc.scalar.activation(out=lnd2, in_=d2c, func=AF.Ln, scale=1.0)
            cond = small.tile([1, BC], F32, tag="cond")
            nc.vector.tensor_scalar(
                out=cond, in0=d1c, scalar1=SWITCH, scalar2=None,
                op0=AluOpType.is_gt,
            )
            condn = small.tile([1, BC], F32, tag="condn")
            nc.vector.tensor_scalar(
                out=condn, in0=d1c, scalar1=SWITCH, scalar2=None,
                op0=AluOpType.is_le,
            )
            lnT = small.tile([1, BC], F32, tag="lnT")
            nc.vector.tensor_tensor(
                out=lnT, in0=lnacc["F"], in1=lnacc["B"], op=AluOpType.add
            )
            u1 = small.tile([1, BC], F32, tag="u1")
            nc.vector.tensor_tensor(out=u1, in0=lnd1, in1=lnT, op=AluOpType.add)
            u2 = small.tile([1, BC], F32, tag="u2")
            nc.vector.tensor_tensor(out=u2, in0=lnd2, in1=lnT, op=AluOpType.add)
            v1 = small.tile([1, BC], F32, tag="v1")
            nc.scalar.activation(out=v1, in_=u1, func=AF.Copy, scale=-1.0, bias=BIAS1)
            v2 = small.tile([1, BC], F32, tag="v2")
            nc.scalar.activation(out=v2, in_=u2, func=AF.Copy, scale=-1.0, bias=BIAS2)
            # cancellation-safe select: cond*v1 + (1-cond)*v2
            p1 = small.tile([1, BC], F32, tag="p1")
            nc.vector.tensor_tensor(out=p1, in0=cond, in1=v1, op=AluOpType.mult)
            p2 = small.tile([1, BC], F32, tag="p2")
            nc.vector.tensor_tensor(out=p2, in0=condn, in1=v2, op=AluOpType.mult)
            lsb = small.tile([1, BC], F32, tag="lsb")
            nc.vector.tensor_tensor(out=lsb, in0=p1, in1=p2, op=AluOpType.add)
            nc.sync.dma_start(out=loss.ap().rearrange("b o -> o b"), in_=lsb)

    nc.compile()
    _built = nc
    return nc


def _host_prep(y_pred: np.ndarray, y_true: np.ndarray):
    y_true = np.asarray(y_true).astype(np.int64)
    ext = np.full((B, S), BLANK, np.int64)
    ext[:, 1::2] = y_true
    ext_m2 = np.concatenate([np.full((B, 2), -1, np.int64), ext[:, :-2]], axis=1)
    skip = (ext != BLANK) & (ext != ext_m2)  # [B,S]
    g = np.zeros((B, S), np.float32)
    g[:, :-2] = skip[:, 2:].astype(np.float32)  # fwd z-mask: g[s] = skip[s+2]
    sm = skip.astype(np.float32)  # bwd w-mask

    # ap_gather indices: 112 per example (97 labels + pad 0), wrapped so
    # index i sits at partition i%16, col i//16, replicated across the 8
    # groups of 16 partitions; examples side by side -> [128, BC*7] per core.
    idxs = np.zeros((B, 112), np.int16)
    idxs[:, :S] = ext.astype(np.int16)
    wrapped = np.zeros((B, 16, 7), np.int16)
    ii = np.arange(112)
    wrapped[:, ii % 16, ii // 16] = idxs
    tiled = np.tile(wrapped, (1, 8, 1))  # [B, 128, 7]

    in_maps = []
    for core in range(N_CORES):
        sl_ = slice(core * BC, (core + 1) * BC)
        in_maps.append(
            {
                "y_pred": np.ascontiguousarray(y_pred[sl_]).astype(np.float32),
                "ext_i16": np.ascontiguousarray(
                    np.concatenate(
                        [tiled[sl_], np.zeros((BC, 128, 1), np.int16)], axis=2
                    ).transpose(1, 0, 2).reshape(128, BC * 8)
                ),
                "gf": np.ascontiguousarray(g[sl_].T),
                "gsm": np.ascontiguousarray(sm[sl_].T),
            }
        )
    return in_maps


def kernel(y_pred: np.ndarray, y_true: np.ndarray) -> np.ndarray:
    nc = _build()
    in_maps = _host_prep(y_pred, y_true)
    res = run_bass_kernel_spmd(nc, in_maps, core_ids=list(range(N_CORES)))
    out = np.concatenate([r["loss"] for r in res.results], axis=0)
    return out.astype(np.float32)


# revision 33
# speedup vs baseline: 1.0035x; 1.0012x over previous
"""CTC loss (keras ctc_batch_cost semantics, full-length labels) on 8 TRN2 cores.

Pure data parallel: 16 examples/core. Per core:

1. Gather (per example x 128-t chunk): contiguous 512KB y DMA, Pool
   ap_gather picks the 97 extended-label classes per t row (int16 idx
   tile, 16B-aligned per-example slices), one PE transpose puts states
   on partitions, ACT scatters K*(p+eps) into per-chain PPg buffers
   [97, u*32 + half*16 + ex], and the z half is P' * skip-mask (DVE in
   phase A, ACT during the scan to keep the DVE queue clear).  Chunks
   q=0,3 run first; q=1,2 stream + scatter during the early scan (their
   blocks are consumed only from step 128 on).

2. Bidirectional scaled prob-domain scan, two 16-example chains (F =
   forward t=0..255, B = backward t=511..256), f32-safe via per-32-step
   per-example renormalization to TGT=1e30 (scale taken from az(u-2) so
   the reciprocal chain stays off the critical path; every applied
   scale is undone exactly via accumulated ln(c) + compile-time
   constants):
     az = [alpha | alpha*mask],  alpha' = (A0@alpha + Sh2@z) * P'_t
   Per chain per step: two accumulating bf16 matmuls with constant
   banded matrices + ONE fused DVE multiply az' = rep2(PSUM) * PPg[:, u].

3. Meet: dot = sum_s (A0@alpha_255 + Sh2@z_255)[s] * q_256[s] via two
   pre-scaled paths.  All Ln inputs are clamped into the HW table's
   valid domain (~[2^-60, 2^60] -- it returns garbage above ~2^64!) and
   the path select is the cancellation-safe cond*v1 + (1-cond)*v2 (NOT
   v2 + cond*(v1-v2), which loses all mantissa bits when the unselected
   path is garbage).
"""

import sys

for p in ("/opt/trn_rl_repo", "/root/.axon_site/_ro/trn_rl_repo"):
    if p not in sys.path:
        sys.path.insert(0, p)

import numpy as np

import concourse.bass as bass
import concourse.bacc as bacc
import concourse.tile as tile
from concourse import mybir
from concourse.alu_op_type import AluOpType
from concourse.bass_utils import run_bass_kernel_spmd

F32 = mybir.dt.float32
F32R = mybir.dt.float32r
BF16 = mybir.dt.bfloat16
I16 = mybir.dt.int16
AF = mybir.ActivationFunctionType

N_CORES = 8
B, T, C, L = 128, 512, 1024, 48
S = 2 * L + 1  # 97
BC = B // N_CORES  # 16
KS = 512.0
EPS = 1e-7
TGT = 1e30
CSC = 2.0 ** -100
SCL1SQ = 2.0 ** -100
SCL2 = 2.0 ** 15
DSC1 = 2.0 ** 55
SWITCH = 1e-25
RN = 32  # renorm period
TM = T // 2  # 256
BLANK = C - 1
N_RENORM = 2 * (TM // RN - 1)  # 7 fwd + 7 bwd
_BIAS_COMMON = (
    T * np.log(KS) + (N_RENORM + 2) * np.log(TGT) + N_RENORM * np.log(CSC)
)
BIAS1 = float(_BIAS_COMMON + np.log(SCL1SQ) + np.log(DSC1))
BIAS2 = float(_BIAS_COMMON + 2 * np.log(SCL2))

_built = None


def _np_consts():
    a0t = np.zeros((S, S), np.float32)
    for k in range(S):
        a0t[k, k] = 1.0
        if k + 1 < S:
            a0t[k, k + 1] = 1.0
    sh2t = np.zeros((S, S), np.float32)
    for k in range(S - 2):
        sh2t[k, k + 2] = 1.0
    a0b = np.zeros((S, S), np.float32)
    for k in range(S):
        a0b[k, k] = 1.0
        if k - 1 >= 0:
            a0b[k, k - 1] = 1.0
    sh2b = np.zeros((S, S), np.float32)
    for k in range(2, S):
        sh2b[k, k - 2] = 1.0
    e01t = np.zeros((S, 2 * BC), np.float32)
    e01t[0:2, :] = TGT
    e96t = np.zeros((S, 2 * BC), np.float32)
    e96t[S - 2 :, :] = TGT
    ones_col = np.ones((S, 1), np.float32)
    tcol = np.full((1, S), TGT, np.float32)
    ident = np.eye(128, dtype=np.float32)
    return {
        "a0t": a0t, "sh2t": sh2t, "a0b": a0b, "sh2b": sh2b,
        "e01t": e01t, "e96t": e96t,
        "ones_col": ones_col, "tcol": tcol, "ident": ident,
    }


def _build():
    global _built
    if _built is not None:
        return _built

    cdefs = _np_consts()

    nc = bacc.Bacc("TRN2")
    y = nc.dram_tensor("y_pred", [BC, T, C], F32, kind="ExternalInput")
    exti = nc.dram_tensor("ext_i16", [128, BC * 8], I16, kind="ExternalInput")
    gf = nc.dram_tensor("gf", [S, BC], F32, kind="ExternalInput")
    gsm = nc.dram_tensor("gsm", [S, BC], F32, kind="ExternalInput")
    loss = nc.dram_tensor("loss", [BC, 1], F32, kind="ExternalOutput")

    handles = {k: nc.inline_tensor(v, name=f"{k}_c") for k, v in cdefs.items()}

    with tile.TileContext(nc) as tc:
        with (
            tc.tile_pool(name="consts", bufs=1) as consts,
            tc.tile_pool(name="persist", bufs=1) as persist,
            tc.tile_pool(name="ybufs", bufs=3) as ybufs,
            tc.tile_pool(name="gthp", bufs=36) as gthp,
            tc.tile_pool(name="idxp", bufs=2) as idxp,
            tc.tile_pool(name="azp", bufs=4) as azp,
            tc.tile_pool(name="small", bufs=4) as small,
            tc.tile_pool(name="tr_ps", bufs=3, space="PSUM") as tr_ps,
            tc.tile_pool(name="sc_ps", bufs=1, space="PSUM") as sc_ps,
            tc.tile_pool(name="rn_ps", bufs=1, space="PSUM") as rn_ps,
        ):
            cs = {}
            for k, v in cdefs.items():
                cs[k] = consts.tile(list(v.shape), F32, tag=k, name=f"c_{k}")
                nc.sync.dma_start(out=cs[k], in_=handles[k].ap())
            csb = {}
            for k in ("a0t", "sh2t", "a0b", "sh2b", "ones_col"):
                csb[k] = consts.tile(
                    list(cdefs[k].shape), BF16, tag=f"{k}_bf", name=f"cb_{k}"
                )
                nc.scalar.activation(
                    out=csb[k], in_=cs[k], func=AF.Copy, scale=1.0
                )
            masks = {}
            for k, h in (("gf", gf), ("gsm", gsm)):
                masks[k] = consts.tile([S, BC], F32, tag=k, name=f"m_{k}")
                nc.sync.dma_start(out=masks[k], in_=h.ap())

            # PPg buffers: chains F, B -> [97, TM * 32]
            # block u columns: [0:16] = P', [16:32] = P'*mask
            ppg = {
                ch: persist.tile(
                    [S, TM * 2 * BC], F32, tag=f"ppg_{ch}", name=f"ppg_{ch}"
                )
                for ch in ("F", "B")
            }

            y_ap = y.ap()

            # ---------------- gather ----------------
            # Unit (b, q): contiguous 512KB y DMA, Pool ap_gather (112 idxs),
            # one PE transpose to put states on partitions, one ACT strided
            # scatter into PPg (folding K scale + K*eps bias), one DVE
            # mask-mult for the z half.  Chunks q=0,3 run first; q=1,2 are
            # interleaved into the scan (their blocks are needed only from
            # step 128 on).
            idx_all = consts.tile([128, BC * 8], I16, tag="idx_all", name="idx_all")
            nc.sync.dma_start(out=idx_all, in_=exti.ap())

            def dst_ap(buf, col):
                return bass.AP(
                    tensor=buf.tensor,
                    offset=buf.offset + col,
                    ap=[buf.ap[0], [2 * BC, 128]],
                )

            _dma_rr = [0]

            def gather_dma(b, q):
                # round-robin 4 tags (bufs=4 each): 4 independent semaphore
                # streams keep more DMAs in flight than one 16-deep ring
                k = _dma_rr[0] % 8
                _dma_rr[0] += 1
                yt = ybufs.tile([128, C], F32, tag=f"y{k}", name="y_t")
                nc.sync.dma_start(out=yt, in_=y_ap[b, q * 128 : (q + 1) * 128, :])
                return yt

            def gather_gather(b, q, yt):
                gth = gthp.tile([128, 112], F32, tag="gth", name="gth")
                nc.gpsimd.ap_gather(
                    out_ap=gth, in_ap=yt, idxs_ap=idx_all[:, b * 8 : b * 8 + 7],
                    channels=128, num_elems=C, d=1, num_idxs=112,
                )
                return gth

            def gather_scatter(b, q, gth, z_on_dve):
                ptr = tr_ps.tile([S, 128], F32, tag="tr", name="ptr")
                nc.tensor.transpose(ptr, gth[:, 0:S], cs["ident"])
                if q < 2:
                    buf, mk, blk0 = ppg["F"], masks["gf"], q * 128
                else:
                    buf, mk, blk0 = ppg["B"], masks["gsm"], (q - 2) * 128
                d_a = dst_ap(buf, blk0 * 2 * BC + b)
                nc.scalar.activation(
                    out=d_a, in_=ptr, func=AF.Copy, scale=KS, bias=KS * EPS
                )
                # z-half: z = mask * P' (mask is a per-partition 0/1 column).
                # DVE in phase A (idle there); ACT during the scan (keeps the
                # DVE queue clear for scan TTs).
                z_dst = dst_ap(buf, blk0 * 2 * BC + BC + b)
                if z_on_dve:
                    nc.vector.tensor_scalar(
                        out=z_dst, in0=d_a, scalar1=mk[:, b : b + 1],
                        scalar2=None, op0=AluOpType.mult,
                    )
                else:
                    nc.scalar.activation(
                        out=z_dst, in_=d_a, func=AF.Copy,
                        scale=mk[:, b : b + 1],
                    )

            phase_a = [(b, q) for q in (0, 3) for b in range(BC)]
            a_fifo = [(b, q, gather_dma(b, q)) for b, q in phase_a]
            for b, q, yt in a_fifo:
                gather_scatter(b, q, gather_gather(b, q, yt), z_on_dve=False)

            # ---------------- scan ----------------
            chains = ["F", "B"]
            mats = {"F": ("a0t", "sh2t"), "B": ("a0b", "sh2b")}

            def sl(ch, u):
                idx = u if ch == "F" else TM - 1 - u
                return idx * 2 * BC

            az, lnacc = {}, {}
            for ch in chains:
                a0 = azp.tile([S, 2 * BC], BF16, tag=f"az{ch}", name=f"az_{ch}")
                init_mask = cs["e01t"] if ch == "F" else cs["e96t"]
                nc.vector.tensor_tensor(
                    out=a0,
                    in0=ppg[ch][:, sl(ch, 0) : sl(ch, 0) + 2 * BC],
                    in1=init_mask,
                    op=AluOpType.mult,
                )
                az[ch] = a0
                ln0 = small.tile([1, BC], F32, tag=f"ln{ch}", name=f"ln_{ch}")
                nc.vector.memset(ln0, 0.0)
                lnacc[ch] = ln0

            # hoisted renorm state: ppgs tiles prepared two steps early
            pending = {}
            rem_units = [(b, q) for q in (1, 2) for b in range(BC)]
            # q1/q2: DMAs burst-issued now (ybufs ring paces, ~16 in flight);
            # Pool ap_gathers sprinkled over idle early-scan steps; the PE
            # transposes + ACT scatters run as two compact bursts so the PE
            # pipeline restarts only twice.
            inflight = [(b, q, gather_dma(b, q)) for b, q in rem_units]
            gths = [gather_gather(b_, q_, yt_) for b_, q_, yt_ in inflight]
            sc_cur = 0
            for u in range(1, TM):
                if sc_cur < len(inflight) and (u % 3 == 1 or u >= 100):
                    b_, q_, _ = inflight[sc_cur]
                    gather_scatter(b_, q_, gths[sc_cur], z_on_dve=False)
                    sc_cur += 1
                for ch in chains:
                    # ---- this chain's step u ----
                    if u % RN == 0:
                        ppg_in = pending.pop((ch, u))
                    else:
                        o = sl(ch, u)
                        ppg_in = ppg[ch][:, o : o + 2 * BC]
                    psx = sc_ps.tile([S, BC], F32, tag=f"ps{ch}", name=f"ps_{ch}")
                    nc.tensor.matmul(
                        psx, csb[mats[ch][0]], az[ch][:, 0:BC],
                        start=True, stop=False,
                    )
                    nc.tensor.matmul(
                        psx, csb[mats[ch][1]], az[ch][:, BC : 2 * BC],
                        start=False, stop=True,
                    )
                    nw = azp.tile([S, 2 * BC], BF16, tag=f"az{ch}", name=f"aznw_{ch}")
                    rep = bass.AP(
                        tensor=psx.tensor,
                        offset=psx.offset,
                        ap=[psx.ap[0], [0, 2], [1, BC]],
                    )
                    nc.vector.tensor_tensor(
                        out=nw.rearrange("p (r c) -> p r c", r=2),
                        in0=rep,
                        in1=ppg_in.rearrange("p (r c) -> p r c", r=2),
                        op=AluOpType.mult,
                    )
                    az[ch] = nw

                    # ---- prepare renorm for step u+2 from az(u) (stale-ok) ----
                    ur = u + 2
                    if ur % RN == 0 and ur < TM:
                        c_ps = rn_ps.tile([1, BC], F32, tag=f"rn{ch}", name=f"cps_{ch}")
                        nc.tensor.matmul(
                            c_ps, csb["ones_col"], az[ch][:, 0:BC],
                            start=True, stop=True,
                        )
                        r_sb = small.tile([1, BC], F32, tag=f"r{ch}", name=f"r_{ch}")
                        nc.vector.reciprocal(r_sb, c_ps)
                        lnc = small.tile([1, BC], F32, tag=f"lnc{ch}", name=f"lnc_{ch}")
                        nc.scalar.activation(out=lnc, in_=c_ps, func=AF.Ln, scale=CSC)
                        ln_new = small.tile([1, BC], F32, tag=f"ln{ch}", name=f"lnn_{ch}")
                        nc.vector.tensor_tensor(
                            out=ln_new, in0=lnacc[ch], in1=lnc, op=AluOpType.add
                        )
                        lnacc[ch] = ln_new
                        rr_ps = rn_ps.tile([S, BC], F32, tag=f"rn{ch}", name=f"rr_{ch}")
                        nc.tensor.matmul(
                            rr_ps, cs["tcol"], r_sb, start=True, stop=True
                        )
                        ppgs = small.tile([S, 2, BC], F32, tag=f"ppgs{ch}", name=f"ppgs_{ch}")
                        rr_rep = bass.AP(
                            tensor=rr_ps.tensor,
                            offset=rr_ps.offset,
                            ap=[rr_ps.ap[0], [0, 2], [1, BC]],
                        )
                        o = sl(ch, ur)
                        nc.vector.tensor_tensor(
                            out=ppgs,
                            in0=ppg[ch][:, o : o + 2 * BC].rearrange(
                                "p (r c) -> p r c", r=2
                            ),
                            in1=rr_rep,
                            op=AluOpType.mult,
                        )
                        pending[(ch, ur)] = ppgs.rearrange("p r c -> p (r c)")

            # ---------------- meet + finalize ----------------
            comb = sc_ps.tile([S, BC], F32, tag="psF", name="comb")
            nc.tensor.matmul(
                comb, csb["a0t"], az["F"][:, 0:BC], start=True, stop=False
            )
            nc.tensor.matmul(
                comb, csb["sh2t"], az["F"][:, BC : 2 * BC], start=False, stop=True
            )
            qv = az["B"][:, 0:BC]
            # path 1 (shallow): product scaled by 2^-100 (fold onto q)
            q1 = small.tile([S, BC], F32, tag="q1")
            nc.vector.tensor_scalar(
                out=q1, in0=qv, scalar1=SCL1SQ, scalar2=None, op0=AluOpType.mult
            )
            m1 = small.tile([S, BC], BF16, tag="m1")
            nc.vector.tensor_tensor(out=m1, in0=comb, in1=q1, op=AluOpType.mult)
            dot1 = sc_ps.tile([1, BC], F32, tag="psB", name="dot1")
            nc.tensor.matmul(dot1, csb["ones_col"], m1, start=True, stop=True)
            # path 2 (deep): each factor scaled by 2^15 and clamped
            q2 = small.tile([S, BC], F32, tag="q2")
            nc.vector.tensor_scalar(
                out=q2, in0=qv, scalar1=SCL2, scalar2=None, op0=AluOpType.mult
            )
            c2 = small.tile([S, BC], F32, tag="c2")
            nc.vector.tensor_scalar(
                out=c2, in0=comb, scalar1=SCL2, scalar2=None, op0=AluOpType.mult
            )
            m2 = small.tile([S, BC], F32, tag="m2")
            nc.vector.tensor_tensor(out=m2, in0=c2, in1=q2, op=AluOpType.mult)
            m2c = small.tile([S, BC], BF16, tag="m2c")
            nc.vector.tensor_scalar(
                out=m2c, in0=m2, scalar1=1e37, scalar2=None, op0=AluOpType.min
            )
            dot2 = sc_ps.tile([1, BC], F32, tag="psB", name="dot2")
            nc.tensor.matmul(dot2, csb["ones_col"], m2c, start=True, stop=True)

            # clamp both dots so the Ln table input stays in ~[2^-60, 2^60]
            d1c = small.tile([1, BC], F32, tag="d1c")
            nc.vector.tensor_scalar(
                out=d1c, in0=dot1, scalar1=1e-30, scalar2=1e-4,
                op0=AluOpType.max, op1=AluOpType.min,
            )
            d2c = small.tile([1, BC], F32, tag="d2c")
            nc.vector.tensor_scalar(
                out=d2c, in0=dot2, scalar1=1e-17, scalar2=3e15,
                op0=AluOpType.max, op1=AluOpType.min,
            )
            lnd1 = small.tile([1, BC], F32, tag="lnd1")
            nc.scalar.activation(out=lnd1, in_=d1c, func=AF.Ln, scale=DSC1)
            lnd2 = small.tile([1, BC], F32, tag="lnd2")
            nc.scalar.activation(out=lnd2, in_=d2c, func=AF.Ln, scale=1.0)
            cond = small.tile([1, BC], F32, tag="cond")
            nc.vector.tensor_scalar(
                out=cond, in0=d1c, scalar1=SWITCH, scalar2=None,
                op0=AluOpType.is_gt,
            )
            condn = small.tile([1, BC], F32, tag="condn")
            nc.vector.tensor_scalar(
                out=condn, in0=d1c, scalar1=SWITCH, scalar2=None,
                op0=AluOpType.is_le,
            )
            lnT = small.tile([1, BC], F32, tag="lnT")
            nc.vector.tensor_tensor(
                out=lnT, in0=lnacc["F"], in1=lnacc["B"], op=AluOpType.add
            )
            u1 = small.tile([1, BC], F32, tag="u1")
            nc.vector.tensor_tensor(out=u1, in0=lnd1, in1=lnT, op=AluOpType.add)
            u2 = small.tile([1, BC], F32, tag="u2")
            nc.vector.tensor_tensor(out=u2, in0=lnd2, in1=lnT, op=AluOpType.add)
            v1 = small.tile([1, BC], F32, tag="v1")
            nc.scalar.activation(out=v1, in_=u1, func=AF.Copy, scale=-1.0, bias=BIAS1)
            v2 = small.tile([1, BC], F32, tag="v2")
            nc.scalar.activation(out=v2, in_=u2, func=AF.Copy, scale=-1.0, bias=BIAS2)
            # cancellation-safe select: cond*v1 + (1-cond)*v2
            p1 = small.tile([1, BC], F32, tag="p1")
            nc.vector.tensor_tensor(out=p1, in0=cond, in1=v1, op=AluOpType.mult)
            p2 = small.tile([1, BC], F32, tag="p2")
            nc.vector.tensor_tensor(out=p2, in0=condn, in1=v2, op=AluOpType.mult)
            lsb = small.tile([1, BC], F32, tag="lsb")
            nc.vector.tensor_tensor(out=lsb, in0=p1, in1=p2, op=AluOpType.add)
            nc.sync.dma_start(out=loss.ap().rearrange("b o -> o b"), in_=lsb)

    nc.compile()
    _built = nc
    return nc


def _host_prep(y_pred: np.ndarray, y_true: np.ndarray):
    y_true = np.asarray(y_true).astype(np.int64)
    ext = np.full((B, S), BLANK, np.int64)
    ext[:, 1::2] = y_true
    ext_m2 = np.concatenate([np.full((B, 2), -1, np.int64), ext[:, :-2]], axis=1)
    skip = (ext != BLANK) & (ext != ext_m2)  # [B,S]
    g = np.zeros((B, S), np.float32)
    g[:, :-2] = skip[:, 2:].astype(np.float32)  # fwd z-mask: g[s] = skip[s+2]
    sm = skip.astype(np.float32)  # bwd w-mask

    # ap_gather indices: 112 per example (97 labels + pad 0), wrapped so
    # index i sits at partition i%16, col i//16, replicated across the 8
    # groups of 16 partitions; examples side by side -> [128, BC*7] per core.
    idxs = np.zeros((B, 112), np.int16)
    idxs[:, :S] = ext.astype(np.int16)
    wrapped = np.zeros((B, 16, 7), np.int16)
    ii = np.arange(112)
    wrapped[:, ii % 16, ii // 16] = idxs
    tiled = np.tile(wrapped, (1, 8, 1))  # [B, 128, 7]

    in_maps = []
    for core in range(N_CORES):
        sl_ = slice(core * BC, (core + 1) * BC)
        in_maps.append(
            {
                "y_pred": np.ascontiguousarray(y_pred[sl_]).astype(np.float32),
                "ext_i16": np.ascontiguousarray(
                    np.concatenate(
                        [tiled[sl_], np.zeros((BC, 128, 1), np.int16)], axis=2
                    ).transpose(1, 0, 2).reshape(128, BC * 8)
                ),
                "gf": np.ascontiguousarray(g[sl_].T),
                "gsm": np.ascontiguousarray(sm[sl_].T),
            }
        )
    return in_maps


def kernel(y_pred: np.ndarray, y_true: np.ndarray) -> np.ndarray:
    nc = _build()
    in_maps = _host_prep(y_pred, y_true)
    res = run_bass_kernel_spmd(nc, in_maps, core_ids=list(range(N_CORES)))
    out = np.concatenate([r["loss"] for r in res.results], axis=0)
    return out.astype(np.float32)
